# revision 1
# baseline (speedup 1.0000x reference)
"""BRPConvEmbedding (3-layer GraphConv + AvgPool readout) on 8 Trainium2 cores.

Sharding: graphs are split contiguously across cores (32 graphs/core), so
pooling is core-local and the output is a pure concat. Each core owns the
nodes of its graphs; within a core, nodes are permuted into dst-groups of 64
nodes whose total in-degree per src-half is capped at 512 (4 chunks of 128
edge slots) via greedy bin-packing, which makes the per-group edge-chunk
layout uniform across all cores (single SPMD program).

Per layer: hn rows are fetched with SWDGE dma_gather (int16 indices; the node
table is split into two halves so indices fit in int16), the per-edge one-hot
is built on the VectorE (iota + tensor_tensor is_equal), the segment-sum runs
on the TensorE (lhsT=gathered chunk, rhs=onehot, PSUM accumulation), followed
by agg.T @ W + fused epilogue, and an AllGather of the new node features.
"""
import numpy as np
from contextlib import ExitStack

import concourse.bacc as bacc
import concourse.mybir as mybir
from concourse import tile
from concourse.bass_utils import run_bass_kernel_spmd

N_NODES = 50000
N_EDGES = 800000
D = 128
N_LAYERS = 3
N_GRAPHS = 256
NCORES = 8
GSZ = 64                  # dst nodes per group
CHUNKS_PER_HALF = 4       # 4*128 = 512 edge-slot cap per (group, half)
CAP = CHUNKS_PER_HALF * 128
GPC = N_GRAPHS // NCORES  # graphs per core


# ----------------------------------------------------------------- host prep
def _pack_groups(nodes, dA, dB):
    """Greedy bin-packing of nodes into groups of <= GSZ nodes with
    sum(dA) <= CAP and sum(dB) <= CAP per group. Returns group id per node."""
    order = np.argsort(-np.maximum(dA, dB), kind="stable")
    gids = np.full(len(nodes), -1, dtype=np.int64)
    usedA, usedB, usedN = [], [], []
    for i in order:
        a, b = dA[i], dB[i]
        best, best_fit = -1, -1.0
        for g in range(len(usedA)):
            if usedN[g] < GSZ and usedA[g] + a <= CAP and usedB[g] + b <= CAP:
                # best-fit: prefer the fullest group that still fits
                fit = max((usedA[g] + a) / CAP, (usedB[g] + b) / CAP)
                if fit > best_fit:
                    best, best_fit = g, fit
        if best < 0:
            usedA.append(0), usedB.append(0), usedN.append(0)
            best = len(usedA) - 1
        gids[i] = best
        usedA[best] += a
        usedB[best] += b
        usedN[best] += 1
    return gids, len(usedA)


def preprocess(feats, W, b, src, dst, graph_ids):
    src = np.asarray(src).astype(np.int64)
    dst = np.asarray(dst).astype(np.int64)
    graph_ids = np.asarray(graph_ids).astype(np.int64)
    feats = np.asarray(feats, dtype=np.float32)

    deg_out = np.maximum(np.bincount(src, minlength=N_NODES), 1).astype(np.float32)
    deg_in = np.maximum(np.bincount(dst, minlength=N_NODES), 1).astype(np.float32)

    node_core = graph_ids // GPC                      # node -> core
    src_half = (node_core[src] >= NCORES // 2).astype(np.int64)
    dA = np.bincount(dst[src_half == 0], minlength=N_NODES)
    dB = np.bincount(dst[src_half == 1], minlength=N_NODES)

    # pack nodes into groups per core
    core_nodes = [np.nonzero(node_core == c)[0] for c in range(NCORES)]
    packs = []
    Gmax = 0
    for c in range(NCORES):
        n = core_nodes[c]
        g, ng = _pack_groups(n, dA[n], dB[n])
        packs.append(g)
        Gmax = max(Gmax, ng)
    G = -(-Gmax // 4) * 4                             # multiple of 4 (supers of 4 groups)
    P = G // 2                                        # pairs (128-node tiles)
    NSUP = G // 4
    SH = G * GSZ                                      # rows per core shard
    R_half = (NCORES // 2) * SH
    assert R_half <= 32767, f"int16 overflow: {R_half}"

    # node -> row
    row = np.full(N_NODES, -1, dtype=np.int64)
    slot_in_group = np.zeros(N_NODES, dtype=np.int64)
    for c in range(NCORES):
        n = core_nodes[c]
        g = packs[c]
        order = np.lexsort((n, g))                    # stable by group
        n_sorted, g_sorted = n[order], g[order]
        # slot = rank within group
        slot = np.zeros(len(n), dtype=np.int64)
        _, starts = np.unique(g_sorted, return_index=True)
        for s0, s1 in zip(starts, list(starts[1:]) + [len(n)]):
            slot[s0:s1] = np.arange(s1 - s0)
        row[n_sorted] = c * SH + g_sorted * GSZ + slot
        slot_in_group[n_sorted] = slot

    # global row map for gather indices: 4 blocks (core-group x pair-half)
    # row_g(c, loc) = (c//4)*R_half + q*(R_half//2) + (c%4)*(SH//2) + (loc - q*(SH//2))
    # where q = loc >= SH//2
    loc_all = row - node_core * SH          # local row within shard (valid where row>=0)
    qh = (loc_all >= SH // 2).astype(np.int64)
    row_g = ((node_core // 4) * R_half + qh * (R_half // 2)
             + (node_core % 4) * (SH // 2) + (loc_all - qh * (SH // 2)))

    # per-core edge layout
    e_core = node_core[dst]
    e_group = np.zeros(N_EDGES, dtype=np.int64)
    for c in range(NCORES):
        m = e_core == c
        d_local = dst[m]
        lr = row[d_local] - c * SH
        e_group[m] = lr // GSZ
    e_dslot = (row[dst] % SH) % GSZ
    e_srow = row_g[src] - src_half * R_half           # int16-safe source row

    per_core = []
    for c in range(NCORES):
        m = np.nonzero(e_core == c)[0]
        g, h, sr, dslt = e_group[m], src_half[m], e_srow[m], e_dslot[m]
        order = np.lexsort((sr, h, g))
        g, h, sr, dslt = g[order], h[order], sr[order], dslt[order]
        # rank within (g, h)
        key = g * 2 + h
        rank = np.arange(len(m)) - np.searchsorted(key, key, side="left")
        k = rank // 128                               # chunk within (g,h)
        p = rank % 128
        assert (k < CHUNKS_PER_HALF).all(), "cap exceeded"
        gi = g % 4                                    # group idx in super
        s = g // 4
        c16 = gi * CHUNKS_PER_HALF + k                # chunk col within (super, half)
        j = c16 * 128 + p                             # slot within (super, half)

        # idx arrays [2*NSUP, 16, 128] (then tiled to 128 partitions)
        idx16 = np.zeros((2 * NSUP, 16, 128), dtype=np.int16)
        t = s * 2 + h
        idx16[t, j % 16, j // 16] = sr.astype(np.int16)
        idx_all = np.tile(idx16, (1, 8, 1)).reshape(2 * NSUP, 128, 128)
        idx_2d = idx_all.transpose(1, 0, 2).reshape(128, 2 * NSUP * 128).copy()

        # dst one-hot scalars [128, 2*NSUP*16], -1 for pad slots
        dstv = np.full((128, 2 * NSUP * 16), -1.0, dtype=np.float32)
        dstv[j % 128, t * 16 + c16] = dslt.astype(np.float32)

        # per-pair node scalars [128, P]
        nodes_c = core_nodes[c]
        lr = row[nodes_c] - c * SH
        deg_in_t = np.ones((128, P), dtype=np.float32)
        deg_out_t = np.ones((128, P), dtype=np.float32)
        gid_t = np.full((128, P), -1.0, dtype=np.float32)
        pr = lr // 128
        pp = lr % 128
        deg_in_t[pp, pr] = deg_in[nodes_c]
        deg_out_t[pp, pr] = deg_out[nodes_c]
        gid_t[pp, pr] = (graph_ids[nodes_c] - c * GPC).astype(np.float32)

        counts = np.maximum(
            np.bincount(graph_ids[nodes_c] - c * GPC, minlength=GPC), 1
        ).astype(np.float32).reshape(GPC, 1)

        feats_shard = np.zeros((SH, D), dtype=np.float32)
        feats_shard[lr] = feats[nodes_c]

        per_core.append(dict(
            idx=idx_2d, dstv=dstv, deg_in=deg_in_t, deg_out=deg_out_t,
            gid=gid_t, counts=counts, feats=feats_shard,
        ))

    b_rep = np.broadcast_to(
        np.asarray(b, dtype=np.float32)[None, :, :], (128, N_LAYERS, D)
    ).copy()
    meta = dict(G=G, P=P, NSUP=NSUP, SH=SH, R_half=R_half)
    shared = dict(W=np.ascontiguousarray(np.asarray(W, dtype=np.float32).transpose(1, 0, 2)),
                  b_rep=b_rep,
                  scr=np.zeros((NCORES * SH, D), dtype=np.float32))
    return per_core, shared, meta


# ------------------------------------------------------------- device build
def build(meta, rep=1, no_coll=False, no_gather=False, split_gather=2):
    G, P, NSUP, SH = meta["G"], meta["P"], meta["NSUP"], meta["SH"]
    R_half = meta["R_half"]
    CH = CHUNKS_PER_HALF
    f32 = mybir.dt.float32

    nc = bacc.Bacc("TRN2", target_bir_lowering=False, debug=False,
                   num_devices=NCORES, dynamic_dma_scratch_size=16384)

    idx_t = nc.dram_tensor("idx", [128, 2 * NSUP * 128], mybir.dt.int16, kind="ExternalInput")
    dstv_t = nc.dram_tensor("dstv", [128, 2 * NSUP * 16], f32, kind="ExternalInput")
    degi_t = nc.dram_tensor("deg_in", [128, P], f32, kind="ExternalInput")
    dego_t = nc.dram_tensor("deg_out", [128, P], f32, kind="ExternalInput")
    gid_t = nc.dram_tensor("gid", [128, P], f32, kind="ExternalInput")
    cnt_t = nc.dram_tensor("counts", [GPC, 1], f32, kind="ExternalInput")
    feats_t = nc.dram_tensor("feats", [SH, D], f32, kind="ExternalInput")
    W_t = nc.dram_tensor("W", [128, N_LAYERS, D], f32, kind="ExternalInput")
    brep_t = nc.dram_tensor("b_rep", [128, N_LAYERS, D], f32, kind="ExternalInput")
    scr_t = [nc.dram_tensor(f"scr{i}", [NCORES * SH, D], f32, kind="ExternalInput")
             for i in range(2)]
    out_t = nc.dram_tensor("out", [GPC, D], f32, kind="ExternalOutput")

    HSH = SH // 2
    hn_part = [
        [nc.dram_tensor(f"hn_p{i}_{q}", [NCORES * HSH, D], f32,
                        kind="Internal", addr_space="Shared") for q in range(2)]
        for i in range(N_LAYERS)
    ]

    def ag_and_copy(nc, hn_shard, l):
        """AllGather hn_shard (split by pair-halves) into scr[l % 2]."""
        scr = scr_t[l % 2]
        for q in range(2):
            nc.gpsimd.collective_compute(
                "AllGather", mybir.AluOpType.bypass,
                replica_groups=[list(range(NCORES))],
                ins=[hn_shard[q * HSH:(q + 1) * HSH, :].opt()],
                outs=[hn_part[l][q].ap().opt()],
            )
            half_blk = (NCORES // 2) * HSH
            for cg in range(2):
                dst0 = cg * (NCORES // 2) * SH + q * half_blk
                nc.sync.dma_start(
                    scr.ap()[dst0:dst0 + half_blk, :],
                    hn_part[l][q].ap()[cg * half_blk:(cg + 1) * half_blk, :],
                )

    with tile.TileContext(nc) as tc, ExitStack() as ctx:
        dram = ctx.enter_context(tc.tile_pool(name="dram", bufs=1, space="DRAM"))
        stat = ctx.enter_context(tc.tile_pool(name="stat", bufs=1))
        gpool = ctx.enter_context(tc.tile_pool(name="gath", bufs=4))
        opool = ctx.enter_context(tc.tile_pool(name="oh", bufs=4))
        spool = ctx.enter_context(tc.tile_pool(name="sb", bufs=4))
        ppool = ctx.enter_context(tc.tile_pool(name="agg_ps", bufs=4, space="PSUM"))
        hpool = ctx.enter_context(tc.tile_pool(name="h_ps", bufs=2, space="PSUM"))
        plpool = ctx.enter_context(tc.tile_pool(name="pool_ps", bufs=1, space="PSUM"))

        hn_shard = dram.tile([SH, D], f32)

        # ---- statics
        idx_sb = stat.tile([128, 2 * NSUP * 128], mybir.dt.int16)
        nc.sync.dma_start(idx_sb[:], idx_t.ap())
        dstv_sb = stat.tile([128, 2 * NSUP * 16], f32)
        nc.sync.dma_start(dstv_sb[:], dstv_t.ap())
        W_sb = stat.tile([128, N_LAYERS, D], f32)
        nc.sync.dma_start(W_sb[:], W_t.ap())
        brep_sb = stat.tile([128, N_LAYERS, D], f32)
        nc.sync.dma_start(brep_sb[:], brep_t.ap())
        gid_sb = stat.tile([128, P], f32)
        nc.sync.dma_start(gid_sb[:], gid_t.ap())
        cnt_sb = stat.tile([GPC, 1], f32)
        nc.sync.dma_start(cnt_sb[:], cnt_t.ap())

        degi_sb = stat.tile([128, P], f32)
        nc.sync.dma_start(degi_sb[:], degi_t.ap())
        dego_sb = stat.tile([128, P], f32)
        nc.sync.dma_start(dego_sb[:], dego_t.ap())
        ni_sb = stat.tile([128, P], f32)   # rsqrt(deg_in)
        no_sb = stat.tile([128, P], f32)   # rsqrt(deg_out)
        nc.vector.reciprocal(ni_sb[:], degi_sb[:])
        nc.scalar.activation(ni_sb[:], ni_sb[:], mybir.ActivationFunctionType.Sqrt)
        nc.vector.reciprocal(no_sb[:], dego_sb[:])
        nc.scalar.activation(no_sb[:], no_sb[:], mybir.ActivationFunctionType.Sqrt)
        rc_sb = stat.tile([GPC, 1], f32)   # 1/counts
        nc.vector.reciprocal(rc_sb[:], cnt_sb[:])

        iota16 = stat.tile([128, GSZ], mybir.dt.int16)
        nc.gpsimd.iota(iota16[:], pattern=[[1, GSZ]], base=0, channel_multiplier=0)
        iota_f = stat.tile([128, GSZ], f32)
        nc.vector.tensor_copy(iota_f[:], iota16[:])

        # graph one-hot [128, P, GPC] (built once; pooling uses layer-2 h)
        groh = stat.tile([128, P, GPC], f32)
        nc.vector.tensor_tensor(
            out=groh[:],
            in0=iota_f[:, :GPC].unsqueeze(1).broadcast_to([128, P, GPC]),
            in1=gid_sb[:].unsqueeze(2).broadcast_to([128, P, GPC]),
            op=mybir.AluOpType.is_equal,
        )

        for _ in range(rep):
            # ---- layer 0 input: hn0 = feats * norm_out
            for pr in range(P):
                ft = spool.tile([128, D], f32, tag="ft")
                nc.sync.dma_start(ft[:], feats_t.ap()[pr * 128:(pr + 1) * 128, :])
                hn0 = spool.tile([128, D], f32, tag="hn")
                nc.vector.tensor_scalar_mul(hn0[:], ft[:], no_sb[:, pr:pr + 1])
                nc.sync.dma_start(hn_shard[pr * 128:(pr + 1) * 128, :], hn0[:])
            if not no_coll:
                ag_and_copy(nc, hn_shard, 0)

            pool_ps = plpool.tile([GPC, D], f32)

            for l in range(N_LAYERS):
                for s in range(NSUP):
                    gA = gpool.tile([128, 4 * CH, D], f32, tag="gA")
                    gB = gpool.tile([128, 4 * CH, D], f32, tag="gB")
                    if split_gather > 1:
                        NP = split_gather          # pieces per half
                        NH = 4 * CH * 128 // NP
                        CPP = 4 * CH // NP         # chunks per piece
                        SCOL = 128 // NP           # idx cols per piece
                        for hh, gt in ((0, gA), (1, gB)):
                            base = scr_t[l % 2].ap()[0:R_half, :] if hh == 0 \
                                else scr_t[l % 2].ap()[R_half:, :]
                            for piece in range(NP):
                                nc.gpsimd.dma_gather(
                                    out_ap=gt[:, piece * CPP:(piece + 1) * CPP, :],
                                    in_ap=base,
                                    idxs_ap=idx_sb[:, (2 * s + hh) * 128 + piece * SCOL:
                                                   (2 * s + hh) * 128 + (piece + 1) * SCOL],
                                    num_idxs=NH, num_idxs_reg=NH,
                                    elem_size=D, single_packet=False)
                    elif not no_gather:
                        nc.gpsimd.dma_gather(
                            out_ap=gA[:], in_ap=scr_t[l % 2].ap()[0:R_half, :],
                            idxs_ap=idx_sb[:, (2 * s) * 128:(2 * s + 1) * 128],
                            num_idxs=4 * CH * 128, num_idxs_reg=4 * CH * 128,
                            elem_size=D, single_packet=False,
                        )
                        nc.gpsimd.dma_gather(
                            out_ap=gB[:], in_ap=scr_t[l % 2].ap()[R_half:, :],
                            idxs_ap=idx_sb[:, (2 * s + 1) * 128:(2 * s + 2) * 128],
                            num_idxs=4 * CH * 128, num_idxs_reg=4 * CH * 128,
                            elem_size=D, single_packet=False,
                        )
                    ohA = opool.tile([128, 4 * CH, GSZ], f32, tag="ohA")
                    ohB = opool.tile([128, 4 * CH, GSZ], f32, tag="ohB")
                    nc.vector.tensor_tensor(
                        out=ohA[:],
                        in0=iota_f[:].unsqueeze(1).broadcast_to([128, 4 * CH, GSZ]),
                        in1=dstv_sb[:, (2 * s) * 16:(2 * s) * 16 + 16]
                            .unsqueeze(2).broadcast_to([128, 4 * CH, GSZ]),
                        op=mybir.AluOpType.is_equal,
                    )
                    nc.vector.tensor_tensor(
                        out=ohB[:],
                        in0=iota_f[:].unsqueeze(1).broadcast_to([128, 4 * CH, GSZ]),
                        in1=dstv_sb[:, (2 * s + 1) * 16:(2 * s + 1) * 16 + 16]
                            .unsqueeze(2).broadcast_to([128, 4 * CH, GSZ]),
                        op=mybir.AluOpType.is_equal,
                    )
                    for pi in range(2):         # pairs in super
                        pr = s * 2 + pi
                        agg = ppool.tile([128, 128], f32, tag="agg")
                        for gj in range(2):     # groups in pair
                            gi = pi * 2 + gj
                            off = gj * GSZ
                            for k in range(CH):
                                nc.tensor.matmul(
                                    agg[:, off:off + GSZ],
                                    gA[:, gi * CH + k, :],
                                    ohA[:, gi * CH + k, :],
                                    start=(k == 0), stop=False,
                                    skip_group_check=True,
                                )
                            for k in range(CH):
                                nc.tensor.matmul(
                                    agg[:, off:off + GSZ],
                                    gB[:, gi * CH + k, :],
                                    ohB[:, gi * CH + k, :],
                                    start=False, stop=(k == CH - 1),
                                    skip_group_check=True,
                                )
                        agg_sb = spool.tile([128, 128], f32, tag="aggsb")
                        nc.scalar.copy(agg_sb[:], agg[:])
                        hps = hpool.tile([128, D], f32, tag="hps")
                        nc.tensor.matmul(hps[:], agg_sb[:], W_sb[:, l, :],
                                         start=True, stop=True)
                        t_sb = spool.tile([128, D], f32, tag="tsb")
                        nc.vector.scalar_tensor_tensor(
                            out=t_sb[:], in0=hps[:], scalar=ni_sb[:, pr:pr + 1],
                            in1=brep_sb[:, l, :],
                            op0=mybir.AluOpType.mult, op1=mybir.AluOpType.add,
                        )
                        if l < N_LAYERS - 1:
                            hn = spool.tile([128, D], f32, tag="hn2")
                            nc.vector.tensor_scalar(
                                out=hn[:], in0=t_sb[:],
                                scalar1=0.0, scalar2=no_sb[:, pr:pr + 1],
                                op0=mybir.AluOpType.max, op1=mybir.AluOpType.mult,
                            )
                            nc.sync.dma_start(
                                hn_shard[pr * 128:(pr + 1) * 128, :], hn[:])
                        else:
                            h_sb = spool.tile([128, D], f32, tag="hsb")
                            nc.vector.tensor_scalar_max(h_sb[:], t_sb[:], 0.0)
                            nc.tensor.matmul(
                                pool_ps[:], groh[:, pr, :], h_sb[:],
                                start=(pr == 0), stop=(pr == P - 1),
                            )
                if l < N_LAYERS - 1 and not no_coll:
                    ag_and_copy(nc, hn_shard, l + 1)

            pool_sb = spool.tile([GPC, D], f32, tag="poolsb")
            nc.vector.tensor_scalar_mul(pool_sb[:], pool_ps[:], rc_sb[:])
            nc.sync.dma_start(out_t.ap(), pool_sb[:])

    nc.compile()
    return nc


def make_in_maps(per_core, shared):
    in_maps = []
    for c in range(NCORES):
        pc = per_core[c]
        in_maps.append({
            "idx": pc["idx"], "dstv": pc["dstv"], "deg_in": pc["deg_in"],
            "deg_out": pc["deg_out"], "gid": pc["gid"], "counts": pc["counts"],
            "feats": pc["feats"], "W": shared["W"], "b_rep": shared["b_rep"],
            "scr0": shared["scr"], "scr1": shared["scr"],
        })
    return in_maps


def kernel(**inputs) -> np.ndarray:
    per_core, shared, meta = preprocess(**inputs)
    nc = build(meta, rep=1)
    in_maps = make_in_maps(per_core, shared)
    res = run_bass_kernel_spmd(nc, in_maps, core_ids=list(range(NCORES)))
    return np.concatenate([res.results[c]["out"] for c in range(NCORES)], axis=0)



# revision 3
# speedup vs baseline: 2.8986x; 2.8986x over previous
"""BRPConvEmbedding (3-layer GraphConv + AvgPool readout) on 8 Trainium2 cores.

v2: super-based layout in bf16 with overlapped AllGathers.

Sharding: graphs split contiguously across cores (32/core); each core owns its
graphs' nodes. Nodes are pre-assigned a row-half bit, then packed per
(core, half) into supers of 256 nodes whose in-edges, split by the source
node's half bit, fit CHT chunks of 128 edge slots per (super, src-half).

Per layer: the per-edge source rows are fetched with one SWDGE dma_gather per
(super, src-half) from a bf16 node table (layer 0's table is expanded on the
host and streamed with HWDGE instead of gathered); the per-edge one-hot
[slots, 256] is built on the DVE (iota + is_equal); the segment-sum runs on
TensorE (lhsT=gathered chunk, rhs=one-hot, PSUM accumulation over chunks);
then agg.T @ W + fused epilogue. Node tables for layers 1-2 are produced by
two bf16 AllGathers per layer (one per row-half), fired as soon as the
producing half's supers finish so they overlap the remaining compute; the
gathers read the AllGather outputs directly (no repack copy).
"""
import numpy as np
from contextlib import ExitStack

import concourse.bacc as bacc
import concourse.mybir as mybir
from concourse import tile
from concourse.bass_utils import run_bass_kernel_spmd

BF16 = mybir.dt.np(mybir.dt.bfloat16)

N_NODES = 50000
N_EDGES = 800000
D = 128
N_LAYERS = 3
N_GRAPHS = 256
NCORES = 8
SSZ = 256                 # dst nodes per super
GPC = N_GRAPHS // NCORES  # graphs per core


# ----------------------------------------------------------------- host prep
def _pack_supers(dA, dB, cap_e):
    """Greedy best-fit-decreasing packing of nodes into supers of <= SSZ nodes
    with sum(dA) <= cap_e and sum(dB) <= cap_e. Returns super id per node."""
    order = np.argsort(-(dA + dB), kind="stable")
    used_n, used_a, used_b = [], [], []
    assign = np.empty(len(dA), dtype=np.int64)
    for i in order:
        a, b = dA[i], dB[i]
        best, best_fit = -1, -1.0
        for j in range(len(used_n)):
            if used_n[j] < SSZ and used_a[j] + a <= cap_e and used_b[j] + b <= cap_e:
                fit = max((used_a[j] + a) / cap_e, (used_b[j] + b) / cap_e)
                if fit > best_fit:
                    best, best_fit = j, fit
        if best < 0:
            used_n.append(0), used_a.append(0), used_b.append(0)
            best = len(used_n) - 1
        assign[i] = best
        used_n[best] += 1
        used_a[best] += a
        used_b[best] += b
    return assign, len(used_n)


def preprocess(feats, W, b, src, dst, graph_ids):
    src = np.asarray(src).astype(np.int64)
    dst = np.asarray(dst).astype(np.int64)
    graph_ids = np.asarray(graph_ids).astype(np.int64)
    feats = np.asarray(feats, dtype=np.float32)

    deg_out = np.maximum(np.bincount(src, minlength=N_NODES), 1).astype(np.float32)
    deg_in = np.maximum(np.bincount(dst, minlength=N_NODES), 1).astype(np.float32)
    norm_out = 1.0 / np.sqrt(deg_out)
    norm_in = 1.0 / np.sqrt(deg_in)

    node_core = graph_ids // GPC

    # ---- row-half assignment: per core, alternate by descending in-degree
    half = np.zeros(N_NODES, dtype=np.int64)
    core_nodes = []
    for c in range(NCORES):
        n = np.nonzero(node_core == c)[0]
        core_nodes.append(n)
        order = np.argsort(-deg_in[n], kind="stable")
        half[n[order[0::2]]] = 0
        half[n[order[1::2]]] = 1

    # ---- per (core, half) super packing over candidate CHT values
    src_half = half[src]
    dA = np.bincount(dst[src_half == 0], minlength=N_NODES)
    dB = np.bincount(dst[src_half == 1], minlength=N_NODES)

    best = None
    for CHT in (15, 16, 17, 18):
        packs, ns_max, ok = {}, 0, True
        for c in range(NCORES):
            for H in (0, 1):
                n = core_nodes[c][half[core_nodes[c]] == H]
                assign, ns = _pack_supers(dA[n], dB[n], CHT * 128)
                packs[(c, H)] = (n, assign)
                ns_max = max(ns_max, ns)
        if 8 * ns_max * SSZ > 32767:
            continue
        slots = ns_max * 2 * CHT
        if best is None or slots < best[0]:
            best = (slots, CHT, ns_max, packs)
    assert best is not None
    _, CHT, NSUP_H, packs = best
    NSUP = 2 * NSUP_H
    HSH = NSUP_H * SSZ        # rows per (core, half)
    P = 2 * NSUP              # pairs (128-row tiles) per core
    RT = NCORES * HSH         # table rows per half
    NI = CHT * 128            # gather slots per (super, src-half)

    # ---- node -> row
    row_local = np.full(N_NODES, -1, dtype=np.int64)   # row within core shard
    for c in range(NCORES):
        for H in (0, 1):
            n, assign = packs[(c, H)]
            order = np.lexsort((n, assign))
            n_s, a_s = n[order], assign[order]
            slot = np.zeros(len(n), dtype=np.int64)
            _, starts = np.unique(a_s, return_index=True)
            for s0, s1 in zip(starts, list(starts[1:]) + [len(n)]):
                slot[s0:s1] = np.arange(s1 - s0)
            row_local[n_s] = (H * NSUP_H + a_s) * SSZ + slot

    # row within the half-table: [core][rows-of-half]
    srow_g = node_core * HSH + (row_local - half * HSH)
    assert srow_g.max() < RT <= 32767

    hn0 = feats * norm_out[:, None]

    # ---- per-core edge layout + tensors
    e_core = node_core[dst]
    e_super = row_local[dst] // SSZ           # global super (0..NSUP-1)
    e_q = src_half                            # src half
    e_dslot = row_local[dst] % SSZ
    e_srow = srow_g[src]

    per_core = []
    for c in range(NCORES):
        m = np.nonzero(e_core == c)[0]
        t = e_super[m] * 2 + e_q[m]
        sr = e_srow[m]
        dslt = e_dslot[m]
        order = np.lexsort((sr, t))
        t, sr, dslt = t[order], sr[order], dslt[order]
        # rank within t
        rank = np.arange(len(m)) - np.searchsorted(t, t, side="left")
        assert rank.max() < NI, f"cap exceeded: {rank.max()} >= {NI}"
        j = rank                                # slot within (super, half)

        # idx array [2*NSUP, 16, NI//16] int16; pads gather row 0 (one-hot
        # zeroes their contribution) -- all-pad or few-pad calls with -1
        # trailing-skip can leave SDMA engines without descriptors and hang
        idx16 = np.zeros((2 * NSUP, 16, NI // 16), dtype=np.int16)
        idx16[t, j % 16, j // 16] = sr.astype(np.int16)
        idx_all = np.tile(idx16, (1, 8, 1)).reshape(2 * NSUP, 128, NI // 16)
        idx_2d = np.ascontiguousarray(
            idx_all.transpose(1, 0, 2).reshape(128, 2 * NSUP * (NI // 16)))

        # dst one-hot scalars [128, 2*NSUP*CHT] bf16, -1 for pad slots
        dstv = np.full((128, 2 * NSUP * CHT), -1.0, dtype=np.float32)
        dstv[j % 128, t * CHT + j // 128] = dslt.astype(np.float32)

        # layer-0 expanded gather stream [128, 2*NSUP*CHT, D] bf16
        t0exp = np.zeros((128, 2 * NSUP * CHT, D), dtype=np.float32)
        t0exp[j % 128, t * CHT + j // 128, :] = hn0[src[m][order]]

        # per-pair node scalars [128, P]
        nodes_c = core_nodes[c]
        lr = row_local[nodes_c]
        ni_t = np.ones((128, P), dtype=np.float32)
        no_t = np.ones((128, P), dtype=np.float32)
        gid_t = np.full((128, P), -1.0, dtype=np.float32)
        ni_t[lr % 128, lr // 128] = norm_in[nodes_c]
        no_t[lr % 128, lr // 128] = norm_out[nodes_c]
        gid_t[lr % 128, lr // 128] = (graph_ids[nodes_c] - c * GPC).astype(np.float32)

        rc = (1.0 / np.maximum(
            np.bincount(graph_ids[nodes_c] - c * GPC, minlength=GPC), 1
        ).astype(np.float32)).reshape(GPC, 1)

        per_core.append(dict(
            idx=idx_2d, dstv=dstv.astype(BF16), t0exp=t0exp.astype(BF16),
            ni=ni_t, no=no_t, gid=gid_t.astype(BF16), rc=rc,
        ))

    shared = dict(
        W=np.ascontiguousarray(
            np.asarray(W, dtype=np.float32).transpose(1, 0, 2)).astype(BF16),
        b_rep=np.broadcast_to(
            np.asarray(b, dtype=np.float32)[None, :, :], (128, N_LAYERS, D)).copy(),
    )
    meta = dict(CHT=CHT, NSUP_H=NSUP_H, NSUP=NSUP, HSH=HSH, P=P, RT=RT, NI=NI)
    return per_core, shared, meta


# ------------------------------------------------------------- device build
def build(meta, rep=1):
    CHT, NSUP_H, NSUP = meta["CHT"], meta["NSUP_H"], meta["NSUP"]
    HSH, P, RT, NI = meta["HSH"], meta["P"], meta["RT"], meta["NI"]
    f32 = mybir.dt.float32
    bf16 = mybir.dt.bfloat16
    IC = NI // 16            # idx cols per (super, half)

    nc = bacc.Bacc("TRN2", target_bir_lowering=False, debug=False,
                   num_devices=NCORES, dynamic_dma_scratch_size=32768,
                   num_swdge_queues=4)

    idx_t = nc.dram_tensor("idx", [128, 2 * NSUP * IC], mybir.dt.int16, kind="ExternalInput")
    dstv_t = nc.dram_tensor("dstv", [128, 2 * NSUP * CHT], bf16, kind="ExternalInput")
    t0exp_t = nc.dram_tensor("t0exp", [128, 2 * NSUP * CHT, D], bf16, kind="ExternalInput")
    ni_t = nc.dram_tensor("ni", [128, P], f32, kind="ExternalInput")
    no_t = nc.dram_tensor("no", [128, P], f32, kind="ExternalInput")
    gid_t = nc.dram_tensor("gid", [128, P], bf16, kind="ExternalInput")
    rc_t = nc.dram_tensor("rc", [GPC, 1], f32, kind="ExternalInput")
    W_t = nc.dram_tensor("W", [128, N_LAYERS, D], bf16, kind="ExternalInput")
    brep_t = nc.dram_tensor("b_rep", [128, N_LAYERS, D], f32, kind="ExternalInput")
    out_t = nc.dram_tensor("out", [GPC, D], f32, kind="ExternalOutput")

    # AllGather outputs: the layer-(l+1) gather tables, one per src-half
    ag_out = [[nc.dram_tensor(f"agout{l}_{q}", [RT, D], bf16,
                              kind="Internal", addr_space="Shared")
               for q in (0, 1)] for l in range(N_LAYERS - 1)]

    with tile.TileContext(nc) as tc, ExitStack() as ctx:
        dram = ctx.enter_context(tc.tile_pool(name="dram", bufs=1, space="DRAM"))
        stat = ctx.enter_context(tc.tile_pool(name="stat", bufs=1))
        gpool = ctx.enter_context(tc.tile_pool(name="gath", bufs=3))
        opool = ctx.enter_context(tc.tile_pool(name="oh", bufs=3))
        spool = ctx.enter_context(tc.tile_pool(name="sb", bufs=4))
        ppool = ctx.enter_context(tc.tile_pool(name="agg_ps", bufs=2, space="PSUM"))
        hpool = ctx.enter_context(tc.tile_pool(name="h_ps", bufs=2, space="PSUM"))
        plpool = ctx.enter_context(tc.tile_pool(name="pool_ps", bufs=1, space="PSUM"))

        # AllGather inputs (per layer, per half)
        hn_half = [[dram.tile([HSH, D], bf16, name=f"hn_half{l}_{q}")
                    for q in (0, 1)] for l in range(N_LAYERS - 1)]

        # ---- statics
        idx_sb = stat.tile([128, 2 * NSUP * IC], mybir.dt.int16)
        nc.sync.dma_start(idx_sb[:], idx_t.ap())
        dstv_sb = stat.tile([128, 2 * NSUP * CHT], bf16)
        nc.sync.dma_start(dstv_sb[:], dstv_t.ap())
        W_sb = stat.tile([128, N_LAYERS, D], bf16)
        nc.sync.dma_start(W_sb[:], W_t.ap())
        brep_sb = stat.tile([128, N_LAYERS, D], f32)
        nc.sync.dma_start(brep_sb[:], brep_t.ap())
        ni_sb = stat.tile([128, P], f32)
        nc.sync.dma_start(ni_sb[:], ni_t.ap())
        no_sb = stat.tile([128, P], f32)
        nc.sync.dma_start(no_sb[:], no_t.ap())
        gid_sb = stat.tile([128, P], bf16)
        nc.sync.dma_start(gid_sb[:], gid_t.ap())
        rc_sb = stat.tile([GPC, 1], f32)
        nc.sync.dma_start(rc_sb[:], rc_t.ap())

        iota16 = stat.tile([128, SSZ], mybir.dt.int16)
        nc.gpsimd.iota(iota16[:], pattern=[[1, SSZ]], base=0, channel_multiplier=0)
        iota_b = stat.tile([128, SSZ], bf16)
        nc.vector.tensor_copy(iota_b[:], iota16[:])

        # graph one-hot [128, P, GPC]
        groh = stat.tile([128, P, GPC], bf16)
        nc.vector.tensor_tensor(
            out=groh[:],
            in0=iota_b[:, :GPC].unsqueeze(1).broadcast_to([128, P, GPC]),
            in1=gid_sb[:].unsqueeze(2).broadcast_to([128, P, GPC]),
            op=mybir.AluOpType.is_equal,
        )

        for _ in range(rep):
            pool_ps = plpool.tile([GPC, D], f32)
            for l in range(N_LAYERS):
                for s in range(NSUP):
                    H = s // NSUP_H
                    g_t = [None, None]
                    oh_t = [None, None]
                    for q in (0, 1):
                        t = s * 2 + q
                        g_t[q] = gpool.tile([128, CHT, D], bf16, tag=f"g{q}", name=f"g{q}")
                        if l == 0:
                            nc.sync.dma_start(
                                g_t[q][:], t0exp_t.ap()[:, t * CHT:(t + 1) * CHT, :])
                        else:
                            # num_idxs > 1024 is broken in the gather ucode;
                            # split into <=8-chunk pieces
                            for i, c0 in enumerate(range(0, CHT, 8)):
                                c1 = min(CHT, c0 + 8)
                                nc.gpsimd.dma_gather(
                                    out_ap=g_t[q][:, c0:c1, :],
                                    in_ap=ag_out[l - 1][q].ap(),
                                    idxs_ap=idx_sb[:, t * IC + c0 * 8:
                                                   t * IC + c1 * 8],
                                    num_idxs=(c1 - c0) * 128,
                                    num_idxs_reg=(c1 - c0) * 128,
                                    elem_size=D, single_packet=False,
                                    queue_num=(q * 2 + s + i) % 4,
                                )
                        oh_t[q] = opool.tile([128, CHT, SSZ], bf16, tag=f"oh{q}", name=f"oh{q}")
                        nc.vector.tensor_tensor(
                            out=oh_t[q][:],
                            in0=iota_b[:].unsqueeze(1).broadcast_to([128, CHT, SSZ]),
                            in1=dstv_sb[:, t * CHT:(t + 1) * CHT]
                                .unsqueeze(2).broadcast_to([128, CHT, SSZ]),
                            op=mybir.AluOpType.is_equal,
                        )
                    agg = ppool.tile([128, SSZ], f32, tag="agg")
                    for q in (0, 1):
                        for k in range(CHT):
                            nc.tensor.matmul(
                                agg[:],
                                g_t[q][:, k, :],
                                oh_t[q][:, k, :],
                                start=(q == 0 and k == 0),
                                stop=(q == 1 and k == CHT - 1),
                                skip_group_check=True,
                            )
                    for pi in (0, 1):
                        pr = s * 2 + pi
                        agg_sb = spool.tile([128, 128], bf16, tag="aggsb")
                        nc.scalar.copy(agg_sb[:], agg[:, pi * 128:(pi + 1) * 128])
                        hps = hpool.tile([128, D], f32, tag="hps")
                        nc.tensor.matmul(hps[:], agg_sb[:], W_sb[:, l, :],
                                         start=True, stop=True)
                        t_sb = spool.tile([128, D], f32, tag="tsb")
                        nc.vector.scalar_tensor_tensor(
                            out=t_sb[:], in0=hps[:], scalar=ni_sb[:, pr:pr + 1],
                            in1=brep_sb[:, l, :],
                            op0=mybir.AluOpType.mult, op1=mybir.AluOpType.add,
                        )
                        if l < N_LAYERS - 1:
                            hn_bf = spool.tile([128, D], bf16, tag="hnb")
                            # relu(t)*no == relu(t*no) since no > 0; ACT is idle
                            nc.scalar.activation(
                                hn_bf[:], t_sb[:],
                                mybir.ActivationFunctionType.Relu,
                                scale=no_sb[:, pr:pr + 1],
                            )
                            r0 = (pr - H * 2 * NSUP_H) * 128
                            nc.sync.dma_start(
                                hn_half[l][H][r0:r0 + 128, :], hn_bf[:])
                        else:
                            h_bf = spool.tile([128, D], bf16, tag="hb")
                            nc.scalar.activation(
                                h_bf[:], t_sb[:],
                                mybir.ActivationFunctionType.Relu)
                            nc.tensor.matmul(
                                pool_ps[:], groh[:, pr, :], h_bf[:],
                                start=(pr == 0), stop=(pr == P - 1),
                            )
                    # fire the AllGather for half 0 as soon as it completes
                    if l < N_LAYERS - 1 and s == NSUP_H - 1:
                        nc.gpsimd.collective_compute(
                            "AllGather", mybir.AluOpType.bypass,
                            replica_groups=[list(range(NCORES))],
                            ins=[hn_half[l][0][:].opt()],
                            outs=[ag_out[l][0].ap().opt()],
                        )
                if l < N_LAYERS - 1:
                    nc.gpsimd.collective_compute(
                        "AllGather", mybir.AluOpType.bypass,
                        replica_groups=[list(range(NCORES))],
                        ins=[hn_half[l][1][:].opt()],
                        outs=[ag_out[l][1].ap().opt()],
                    )

            pool_sb = spool.tile([GPC, D], f32, tag="poolsb")
            nc.vector.tensor_scalar_mul(pool_sb[:], pool_ps[:], rc_sb[:])
            nc.sync.dma_start(out_t.ap(), pool_sb[:])

    nc.compile()
    return nc


def make_in_maps(per_core, shared):
    in_maps = []
    for c in range(NCORES):
        pc = per_core[c]
        in_maps.append({
            "idx": pc["idx"], "dstv": pc["dstv"], "t0exp": pc["t0exp"],
            "ni": pc["ni"], "no": pc["no"], "gid": pc["gid"], "rc": pc["rc"],
            "W": shared["W"], "b_rep": shared["b_rep"],
        })
    return in_maps


def kernel(**inputs) -> np.ndarray:
    per_core, shared, meta = preprocess(**inputs)
    nc = build(meta, rep=1)
    in_maps = make_in_maps(per_core, shared)
    res = run_bass_kernel_spmd(nc, in_maps, core_ids=list(range(NCORES)))
    return np.concatenate([res.results[c]["out"] for c in range(NCORES)], axis=0)
